# revision 1
# baseline (speedup 1.0000x reference)
"""Trainium2 Bass kernel for nn_CP_Based (CP-decomposition interaction layer).

Math (full problem):
    t[b,f,r,u] = sum_d X[b,f,d] * K[d,r,f,u]      (B=1024, F=64, D=4, R=32, U=128)
    had[b,r,u] = prod_f t[b,f,r,u]
    out[b,u]   = sum_r had[b,r,u]

Strategy:
  * Shard batch across 8 cores (B_loc = 128 = one partition tile).
  * Feature-tripling (host-side weight repack): for a triple (f0,f1,f2),
        t3 = t[.,f0,.] * t[.,f1,.] * t[.,f2,.]
           = sum_{d3=0..63} X3[b,j,d3] * K3[d3,r,j,u]
    with X3/K3 outer products of the per-feature slices. One K=64 matmul per
    triple replaces three K=4 matmuls AND cuts the elementwise hadamard from
    63 to 21 multiplies per output element (the DVE is the bottleneck engine:
    fp32 tensor_tensor runs at 1 elem/cycle/partition @ 0.96 GHz).
    64 = 21*3 + 1: factor 21 is the lone feature 63, zero-padded to K=64.
  * PE: 2 factors run concurrently via row tiling (tile_position=(64s,0)),
    each filling a [128,1024] 2-bank psum tile per (r,u) chunk.
  * DVE: running product P[b, r*u] *= psum factor chunks (one PSUM operand
    per op is a HW limit). ScalarE initializes P for the first factor.
  * Final sum over r: strided tensor_reduce.

Host prep is pure input repacking (outer products of the small inputs,
~12M mults vs ~1.3G MACs + 270M multiplies on device).
"""

import numpy as np

B, F, D, R, U = 1024, 64, 4, 32, 128
NCORES = 8
BLOC = B // NCORES          # 128 batch rows per core
NFAC = 22                   # 21 triples + 1 padded single
NGRP = NFAC // 2            # 11 groups of 2 row-tiled factors
D3 = 64                     # contraction dim per triple (4^3)
RU = R * U                  # 4096
CHUNK = 1024                # 2 psum banks per factor-chunk
NCHUNK = RU // CHUNK        # 4

_cached = {}


def _build_nc(n_rep=1, chunk=CHUNK, unroll_reps=False):
    import concourse.bass as bass
    import concourse.mybir as mybir
    import concourse.tile as tile
    from concourse import bacc

    nch = RU // chunk
    nps = 8 // (chunk // 512)  # psum tiles to fill all 8 banks
    fp32 = mybir.dt.float32
    nc = bacc.Bacc("TRN2", target_bir_lowering=False, debug=False)

    xt_d = nc.dram_tensor("xt", [128, NGRP * BLOC], fp32, kind="ExternalInput").ap()
    kr_d = nc.dram_tensor("kr", [NGRP, 128, RU], fp32, kind="ExternalInput").ap()
    out_d = nc.dram_tensor("out", [BLOC, U], fp32, kind="ExternalOutput").ap()

    with tile.TileContext(nc) as tc:
        with (
            tc.tile_pool(name="const", bufs=1) as const_pool,
            tc.tile_pool(name="kt", bufs=3) as kpool,
            tc.tile_pool(name="prod", bufs=1) as ppool,
            tc.tile_pool(name="outp", bufs=1) as opool,
            tc.tile_pool(name="ps", bufs=nps, space="PSUM") as pspool,
        ):
            xt = const_pool.tile([128, NGRP * BLOC], fp32)
            nc.sync.dma_start(xt[:], xt_d[:])

            P = ppool.tile([128, RU], fp32)

            def body():
                for m in range(NGRP):
                    kt = kpool.tile([128, RU], fp32, tag="kt")
                    nc.sync.dma_start(kt[:], kr_d[m])
                    for c in range(nch):
                        sl = slice(c * chunk, (c + 1) * chunk)
                        ps = []
                        for s in range(2):
                            pst = pspool.tile([128, chunk], fp32, tag="ps")
                            for h in range(chunk // 512):
                                hs = slice(h * 512, (h + 1) * 512)
                                nc.tensor.matmul(
                                    pst[:, hs],
                                    xt[64 * s : 64 * s + D3, m * BLOC : (m + 1) * BLOC],
                                    kt[64 * s : 64 * s + D3, c * chunk + h * 512 : c * chunk + (h + 1) * 512],
                                    start=True,
                                    stop=True,
                                    tile_position=(64 * s, 0),
                                )
                            ps.append(pst)
                        # DVE reads at most one PSUM operand per op: chain the
                        # running product through SBUF. Init via ScalarE copy.
                        if m == 0:
                            nc.scalar.copy(P[:, sl], ps[0][:])
                        else:
                            nc.vector.tensor_mul(P[:, sl], P[:, sl], ps[0][:])
                        nc.vector.tensor_mul(P[:, sl], P[:, sl], ps[1][:])

            if n_rep == 1:
                body()
            elif unroll_reps:
                for _ in range(n_rep):
                    body()
            else:
                # benchmarking mode: repeat the (idempotent) body on-device
                with tc.For_i(0, n_rep, 1):
                    body()

            osum = opool.tile([BLOC, U], fp32)
            nc.vector.tensor_reduce(
                osum[:],
                P[:].rearrange("p (r u) -> p u r", r=R),
                axis=mybir.AxisListType.X,
                op=mybir.AluOpType.add,
            )
            nc.sync.dma_start(out_d[:], osum[:])

    nc.compile()
    return nc


def _host_prep(X, K):
    """Repack inputs: per-core X3 outer products + shared K3 outer products.

    Factor j < 21 covers features (3j, 3j+1, 3j+2) with contraction index
    d3 = 16*d0 + 4*d1 + d2; factor 21 is feature 63 (d3 = d, rest zero).
    Packed layouts match SBUF tiles directly:
      kr[m, row, r*U+u]: row = 64*s + d3 holds factor (2m+s).
      xt[row, m*BLOC+b]: same row convention.
    """
    f32 = np.float32
    NT = 21
    fa = [3 * j for j in range(NT)]

    # K3 [j, d3, r*u]
    ka = K[:, :, [3 * j for j in range(NT)], :]      # [4, 32, 21, 128] (d,r,j,u)
    kb = K[:, :, [3 * j + 1 for j in range(NT)], :]
    kc = K[:, :, [3 * j + 2 for j in range(NT)], :]
    K3 = (
        ka[:, None, None] * kb[None, :, None] * kc[None, None, :]
    )                                                # [4,4,4,32,21,128] (d0,d1,d2,r,j,u)
    K3 = K3.transpose(4, 0, 1, 2, 3, 5).reshape(NT, D3, RU)  # [j, d3, r*u]
    K3f = np.zeros((NFAC, D3, RU), dtype=f32)
    K3f[:NT] = K3
    K3f[NT, :D, :] = K[:, :, 63, :].reshape(D, RU)   # lone feature 63
    kr = np.ascontiguousarray(
        K3f.reshape(NGRP, 2, D3, RU).reshape(NGRP, 128, RU)
    )

    # X3 per core [row, m*BLOC+b]
    xts = []
    for c in range(NCORES):
        Xc = X[c * BLOC : (c + 1) * BLOC]            # [128, 64, 4] (b, f, d)
        xa = Xc[:, [3 * j for j in range(NT)], :]    # [b, j, 4]
        xb = Xc[:, [3 * j + 1 for j in range(NT)], :]
        xc = Xc[:, [3 * j + 2 for j in range(NT)], :]
        X3 = (
            xa[:, :, :, None, None] * xb[:, :, None, :, None] * xc[:, :, None, None, :]
        )                                            # [b, j, 4, 4, 4]
        X3 = X3.reshape(BLOC, NT, D3)
        X3f = np.zeros((BLOC, NFAC, D3), dtype=f32)
        X3f[:, :NT] = X3
        X3f[:, NT, :D] = Xc[:, 63, :]
        xt = X3f.transpose(1, 2, 0).reshape(NGRP, 128, BLOC)  # [m, row, b]
        xts.append(np.ascontiguousarray(xt.transpose(1, 0, 2).reshape(128, NGRP * BLOC)))
    return xts, kr


def kernel(**inputs):
    from concourse.bass_utils import run_bass_kernel_spmd

    X = np.asarray(inputs["X"], dtype=np.float32)
    K = np.asarray(inputs["kernel"], dtype=np.float32)
    assert X.shape == (B, F, D) and K.shape == (D, R, F, U)

    if "nc" not in _cached:
        _cached["nc"] = _build_nc()
    nc = _cached["nc"]

    xts, kr = _host_prep(X, K)
    in_maps = [{"xt": xts[c], "kr": kr} for c in range(NCORES)]
    res = run_bass_kernel_spmd(nc, in_maps, core_ids=list(range(NCORES)))
    return np.concatenate([res.results[c]["out"] for c in range(NCORES)], axis=0)



# revision 4
# speedup vs baseline: 1.0758x; 1.0758x over previous
"""Trainium2 Bass kernel for nn_CP_Based (CP-decomposition interaction layer).

Math (full problem):
    t[b,f,r,u] = sum_d X[b,f,d] * K[d,r,f,u]      (B=1024, F=64, D=4, R=32, U=128)
    had[b,r,u] = prod_f t[b,f,r,u]
    out[b,u]   = sum_r had[b,r,u]

Strategy (v2):
  * Shard batch x units across 8 cores as (2 batch halves) x (4 unit quarters):
    per core B_loc=512 (4 partition tiles of 128) and RU_loc = 32r x 32u = 1024
    columns (u-major, r contiguous innermost for a cheap final reduce).
  * Feature grouping (host-side weight repack): 7 QUads of 4 features
    (K=4^4=256, realized as 2 PSUM-accumulated K=128 matmul passes) plus
    12 TRIples of 3 features (K=64, two triples row-tiled per matmul pair
    via tile_position).  19 factor tiles per batch tile instead of 22,
    cutting the PSUM-drain workload (the hard engine bottleneck: only DVE
    at 0.96 GHz and Act at 1.2 GHz can read PSUM).
  * All matmul inputs fp16 (PE runs 1 row/cycle vs 4 for fp32); PSUM fp32.
  * Drain split: 6 tiles fused-multiplied into an fp32 running product on
    DVE (tensor_tensor with one PSUM operand); 13 tiles copied to fp16
    SBUF by the Act engine.  fp16 SBUF products run at DVE 2x mode; the
    GpSimd (Pool) engine takes a 6-factor side chain and the final
    r-reduduction to offload DVE.
  * Weights/activations preloaded to SBUF once (7.9 MB total DMA).
"""

import numpy as np

B, F, D, R, U = 1024, 64, 4, 32, 128
NCORES = 8
BSH, USH = 2, 4                 # batch shards x unit shards
BLOC = B // BSH                 # 512 batch rows per core
NBT = BLOC // 128               # 4 batch tiles of 128
ULOC = U // USH                 # 32 units per core
RUL = R * ULOC                  # 1024 columns (u-major: col = u*32 + r)
NQ, NT = 7, 12                  # quads (feats 0..27), triples (feats 28..63)
NPAIR = NT // 2                 # 6 row-tiled triple pairs
NSLOT = 2 * NQ + NPAIR          # 20 weight slots of [128, RUL]
NTILE = NQ + NT                 # 19 factor tiles per batch tile

# factor-tile consumer assignment (per batch tile); tiles 0..11 triples
# (produced first; their kt arrives first), 12..18 quads.
FUSED = [0, 3, 6, 9, 12, 15]            # DVE fused fp32 chain
ACTS = [i for i in range(NTILE) if i not in FUSED]   # 13 Act drains
DVE_F = ACTS[:7]                        # fp16 chain on DVE (6 muls)
POOL_F = ACTS[7:]                       # 6 factors -> Pool chain (5 muls)

_cached = {}


def _build_nc():
    import concourse.bass as bass
    import concourse.mybir as mybir
    import concourse.tile as tile
    from concourse import bacc

    fp32 = mybir.dt.float32
    fp16 = mybir.dt.float16
    nc = bacc.Bacc("TRN2", target_bir_lowering=False, debug=False)

    xt_d = nc.dram_tensor("xt", [NBT, 128, NSLOT * 128], fp16, kind="ExternalInput").ap()
    kt_d = nc.dram_tensor("kt", [NSLOT, 128, RUL], fp16, kind="ExternalInput").ap()
    out_d = nc.dram_tensor("out", [BLOC, ULOC], fp32, kind="ExternalOutput").ap()

    with tile.TileContext(nc) as tc:
        with (
            tc.tile_pool(name="kt", bufs=1) as ktpool,
            tc.tile_pool(name="xt", bufs=1) as xtpool,
            tc.tile_pool(name="fb", bufs=10) as fbpool,
            tc.tile_pool(name="acc", bufs=2) as accpool,
            tc.tile_pool(name="out", bufs=2) as outpool,
            tc.tile_pool(name="ps", bufs=4, space="PSUM") as pspool,
        ):
            # preload: btile-0 stationary, then kt in consumption order
            # (triple-pair slots first), then remaining stationaries.
            xts = []
            for t in range(NBT):
                xts.append(
                    xtpool.tile([128, NSLOT * 128], fp16, tag=f"xt{t}", name=f"xt{t}")
                )
            nc.sync.dma_start(xts[0][:], xt_d[0])
            kts = [None] * NSLOT
            for s in list(range(2 * NQ, NSLOT)) + list(range(2 * NQ)):
                kts[s] = ktpool.tile([128, RUL], fp16, tag=f"kt{s}", name=f"kt{s}")
                nc.sync.dma_start(kts[s][:], kt_d[s])
            for t in range(1, NBT):
                nc.sync.dma_start(xts[t][:], xt_d[t])

            for t in range(NBT):
                xt = xts[t]

                def xsl(s):
                    return slice(s * 128, (s + 1) * 128)

                # --- produce the 19 factor psum tiles (triples first) ---
                ptiles = []
                pend = []

                def produce(i):
                    if i < NT:  # triple i, pair p = i//2, sub s = i%2
                        p, s = divmod(i, 2)
                        slot = 2 * NQ + p
                        ps = pspool.tile([128, RUL], fp32, tag="ps")
                        rows = slice(64 * s, 64 * s + 64)
                        for h in range(2):
                            cs = slice(512 * h, 512 * h + 512)
                            nc.tensor.matmul(
                                ps[:, cs],
                                xt[rows, xsl(slot)],
                                kts[slot][rows, cs],
                                start=True,
                                stop=True,
                                tile_position=(64 * s, 0),
                            )
                        return ps
                    q = i - NT
                    ps = pspool.tile([128, RUL], fp32, tag="ps")
                    for h in range(2):
                        slot = 2 * q + h
                        for c in range(2):
                            cs = slice(512 * c, 512 * c + 512)
                            nc.tensor.matmul(
                                ps[:, cs],
                                xt[:, xsl(slot)],
                                kts[slot][:, cs],
                                start=(h == 0),
                                stop=(h == 1),
                            )
                    return ps

                P = accpool.tile([128, RUL], fp32, tag="P")
                CA = accpool.tile([128, RUL], fp16, tag="CA")
                CP = accpool.tile([128, RUL], fp16, tag="CP")

                fbs = {}
                nfused = 0
                ndve = 0
                npool = 0
                for i in range(NTILE):
                    ps = produce(i)
                    if i in FUSED:
                        if nfused == 0:
                            nc.vector.tensor_copy(P[:], ps[:])
                        else:
                            nc.vector.tensor_mul(P[:], P[:], ps[:])
                        nfused += 1
                    else:
                        fb = fbpool.tile([128, RUL], fp16, tag="fb")
                        nc.scalar.copy(fb[:], ps[:])
                        fbs[i] = fb
                    # chain consumption as factors become available
                    if i in DVE_F:
                        ndve += 1
                        if ndve == 2:
                            nc.vector.tensor_mul(
                                CA[:], fbs[DVE_F[0]][:], fbs[DVE_F[1]][:]
                            )
                        elif ndve > 2:
                            nc.vector.tensor_mul(CA[:], CA[:], fbs[i][:])
                    elif i in POOL_F:
                        npool += 1
                        if npool == 2:
                            nc.gpsimd.tensor_mul(
                                CP[:], fbs[POOL_F[0]][:], fbs[POOL_F[1]][:]
                            )
                        elif npool > 2:
                            nc.gpsimd.tensor_mul(CP[:], CP[:], fbs[i][:])

                nc.vector.tensor_mul(CA[:], CA[:], CP[:])
                nc.gpsimd.tensor_mul(P[:], P[:], CA[:])

                osum = outpool.tile([128, ULOC], fp32, tag="osum")
                nc.vector.tensor_reduce(
                    osum[:],
                    P[:].rearrange("p (u r) -> p u r", r=R),
                    axis=mybir.AxisListType.X,
                    op=mybir.AluOpType.add,
                )
                nc.sync.dma_start(out_d[t * 128 : (t + 1) * 128, :], osum[:])

    nc.compile()
    return nc


def _host_prep(X, K):
    """Repack inputs into per-core fp16 stationary/moving operands.

    Per core (bi, uj): quads q cover features 4q..4q+3 as two K=128
    psum-accumulated passes (rows = (d0,d1,d2,l), l indexing half of the
    4th feature's d); triples j cover features 28+3j..30+3j with K=64,
    packed two per slot for row-tiled matmuls.  Columns are u-major
    (col = u*32 + r) so the final r-reduction is over contiguous runs.
    """
    f16 = np.float16
    xt_all, kt_all = [], []
    for bi in range(BSH):
        Xc = X[bi * BLOC : (bi + 1) * BLOC]                    # [512, 64, 4]
        xt_all.append(Xc)
    kt_cores = []
    xt_cores = []
    for bi in range(BSH):
        Xc = xt_all[bi]
        for uj in range(USH):
            Ku = K[:, :, :, uj * ULOC : (uj + 1) * ULOC]       # [4,32,64,32]
            # Kf[f, d, col] with col = u*32 + r
            Kf = np.ascontiguousarray(
                Ku.transpose(2, 0, 3, 1).reshape(F, D, RUL)
            )
            kt = np.zeros((NSLOT, 128, RUL), dtype=f16)
            xt = np.zeros((NBT, 128, NSLOT * 128), dtype=f16)
            for q in range(NQ):
                f0 = 4 * q
                K012 = (
                    Kf[f0][:, None, None, :]
                    * Kf[f0 + 1][None, :, None, :]
                    * Kf[f0 + 2][None, None, :, :]
                ).reshape(64, RUL)
                X012 = (
                    Xc[:, f0, :, None, None]
                    * Xc[:, f0 + 1, None, :, None]
                    * Xc[:, f0 + 2, None, None, :]
                ).reshape(BLOC, 64)
                for h in range(2):
                    s = 2 * q + h
                    kt[s] = (
                        K012[:, None, :] * Kf[f0 + 3][2 * h : 2 * h + 2][None, :, :]
                    ).reshape(128, RUL)
                    X4h = (
                        X012[:, :, None] * Xc[:, f0 + 3, 2 * h : 2 * h + 2][:, None, :]
                    ).reshape(BLOC, 128)
                    for t in range(NBT):
                        xt[t, :, s * 128 : (s + 1) * 128] = X4h[
                            t * 128 : (t + 1) * 128
                        ].T
            for j in range(NT):
                f0 = F - 3 * NT + 3 * j
                K3 = (
                    Kf[f0][:, None, None, :]
                    * Kf[f0 + 1][None, :, None, :]
                    * Kf[f0 + 2][None, None, :, :]
                ).reshape(64, RUL)
                X3 = (
                    Xc[:, f0, :, None, None]
                    * Xc[:, f0 + 1, None, :, None]
                    * Xc[:, f0 + 2, None, None, :]
                ).reshape(BLOC, 64)
                p, s = divmod(j, 2)
                slot = 2 * NQ + p
                rows = slice(64 * s, 64 * s + 64)
                kt[slot, rows] = K3
                for t in range(NBT):
                    xt[t, rows, slot * 128 : (slot + 1) * 128] = X3[
                        t * 128 : (t + 1) * 128
                    ].T
            kt_cores.append(np.ascontiguousarray(kt))
            xt_cores.append(np.ascontiguousarray(xt))
    return [
        {"xt": xt_cores[c], "kt": kt_cores[c]} for c in range(NCORES)
    ]


def kernel(**inputs):
    from concourse.bass_utils import run_bass_kernel_spmd

    X = np.asarray(inputs["X"], dtype=np.float32)
    K = np.asarray(inputs["kernel"], dtype=np.float32)
    assert X.shape == (B, F, D) and K.shape == (D, R, F, U)

    if "nc" not in _cached:
        _cached["nc"] = _build_nc()
    nc = _cached["nc"]

    in_maps = _host_prep(X, K)
    res = run_bass_kernel_spmd(nc, in_maps, core_ids=list(range(NCORES)))
    out = np.zeros((B, U), dtype=np.float32)
    for c in range(NCORES):
        bi, uj = divmod(c, USH)
        out[bi * BLOC : (bi + 1) * BLOC, uj * ULOC : (uj + 1) * ULOC] = res.results[
            c
        ]["out"]
    return out


# revision 9
# speedup vs baseline: 1.1337x; 1.0539x over previous
"""Trainium2 Bass kernel for nn_CP_Based (CP-decomposition interaction layer).

Math (full problem):
    t[b,f,r,u] = sum_d X[b,f,d] * K[d,r,f,u]      (B=1024, F=64, D=4, R=32, U=128)
    had[b,r,u] = prod_f t[b,f,r,u]
    out[b,u]   = sum_r had[b,r,u]

Strategy (v4):
  * Shard batch x units across 8 cores as (2 batch halves) x (4 unit
    quarters): per core B_loc=512 (4 partition tiles) and RU_loc = 32r x 32u
    = 1024 columns (u-major, r contiguous innermost for the final reduce).
  * Host-side feature grouping into multilinear factors: the PSUM-read
    bandwidth of the only two PSUM-capable engines (DVE @0.96 GHz, Act
    @1.2 GHz) is the hard roofline, so we minimize factor count.  12 QUADS
    of 4 features (K=4^4=256 as 2 PSUM-accumulated K=128 passes) + 5
    triples (K=64, row-tiled in pairs via tile_position) + feature 63
    alone = 18 factor tiles per batch tile instead of 64 features.
  * All matmul inputs fp16: the PE streams 512-row matmuls at ~214 ns when
    continuously busy (full p-state).  A 4-deep [128,1024] PSUM ring keeps
    it fed.
  * Drains: 5 tiles fold into an fp32 running product P on DVE (fused
    tensor_tensor with one PSUM operand - precision anchor); 13 tiles are
    drained to fp16 SBUF by Act.  DVE chains 8 of those (fp16 muls), Pool
    chains the other 5; final merges + the strided r-reduce on DVE.
"""

import numpy as np

B, F, D, R, U = 1024, 64, 4, 32, 128
NCORES = 8
BSH, USH = 2, 4                 # batch shards x unit shards
BLOC = B // BSH                 # 512 batch rows per core
NBT = BLOC // 128               # 4 batch tiles of 128
ULOC = U // USH                 # 32 units per core
RUL = R * ULOC                  # 1024 columns (u-major: col = u*32 + r)
NQ = 4                          # quads (features 0..15)
NT = 16                         # triples (features 16..63)
NTILE = NT + NQ                 # 20 factor tiles per batch tile
NPAIR = NT // 2                 # triple pairs
NSLOT = NPAIR + 2 * NQ          # kt slots: 8 triple-pairs + 2 per quad

# consumer assignment (tiles 0..5 = triple halves incl lone, 6..17 quads)
FUSED = (0, 3, 6, 9, 12, 15, 18)              # DVE fused fp32 chain (7)
ACTS = tuple(i for i in range(NTILE) if i not in FUSED)   # 13 Act drains
DVE_F = ACTS[:7]                              # fp16 chain on DVE (6 muls)
POOL_F = ACTS[7:]                             # 6 factors -> Pool chain

_cached = {}


def _build_nc():
    import concourse.bass as bass
    import concourse.mybir as mybir
    import concourse.tile as tile
    from concourse import bacc

    fp32 = mybir.dt.float32
    fp16 = mybir.dt.float16
    nc = bacc.Bacc("TRN2", target_bir_lowering=False, debug=False)

    xt_d = nc.dram_tensor("xt", [NBT, 128, NSLOT * 128], fp16, kind="ExternalInput").ap()
    kt_d = nc.dram_tensor("kt", [NSLOT, 128, RUL], fp16, kind="ExternalInput").ap()
    out_d = nc.dram_tensor("out", [BLOC, ULOC], fp32, kind="ExternalOutput").ap()

    with tile.TileContext(nc) as tc:
        with (
            tc.tile_pool(name="kt", bufs=1) as ktpool,
            tc.tile_pool(name="xt", bufs=1) as xtpool,
            tc.tile_pool(name="fb", bufs=10) as fbpool,
            tc.tile_pool(name="acc", bufs=2) as accpool,
            tc.tile_pool(name="out", bufs=2) as outpool,
            tc.tile_pool(name="ps", bufs=4, space="PSUM") as pspool,
        ):
            xts = []
            for t in range(NBT):
                xts.append(
                    xtpool.tile([128, NSLOT * 128], fp16, tag=f"xt{t}", name=f"xt{t}")
                )
            nc.sync.dma_start(xts[0][:], xt_d[0])
            kts = []
            for s in range(NSLOT):
                kts.append(ktpool.tile([128, RUL], fp16, tag=f"kt{s}", name=f"kt{s}"))
                nc.sync.dma_start(kts[s][:], kt_d[s])
            for t in range(1, NBT):
                nc.sync.dma_start(xts[t][:], xt_d[t])

            pending = []

            def xsl(s):
                return slice(s * 128, (s + 1) * 128)

            for t in range(NBT):
                xt = xts[t]
                P = accpool.tile([128, RUL], fp32, tag="P", name="P")
                CA = accpool.tile([128, RUL], fp16, tag="CA", name="CA")
                CP = accpool.tile([128, RUL], fp16, tag="CP", name="CP")
                fbs = {}
                nfused = 0
                ndve = 0
                npool = 0
                for i in range(NTILE):
                    ps = pspool.tile([128, RUL], fp32, tag="ps", name="ps")
                    if i < NT:  # triple half: pair p = i//2, sub s = i%2
                        p, s = divmod(i, 2)
                        rows = slice(64 * s, 64 * s + 64)
                        for h in range(2):
                            cs = slice(512 * h, 512 * h + 512)
                            nc.tensor.matmul(
                                ps[:, cs],
                                xt[rows, xsl(p)],
                                kts[p][rows, cs],
                                start=True,
                                stop=True,
                                tile_position=(64 * s, 0),
                            )
                    else:  # quad: 2 psum-accumulated K=128 passes
                        q = i - NT
                        for h in range(2):
                            slot = NPAIR + 2 * q + h
                            for c in range(2):
                                cs = slice(512 * c, 512 * c + 512)
                                nc.tensor.matmul(
                                    ps[:, cs],
                                    xt[:, xsl(slot)],
                                    kts[slot][:, cs],
                                    start=(h == 0),
                                    stop=(h == 1),
                                )
                    if i in FUSED:
                        nfused += 1
                        if nfused == 1:
                            nc.vector.tensor_copy(P[:], ps[:])
                        else:
                            nc.vector.tensor_mul(P[:], P[:], ps[:])
                    else:
                        fb = fbpool.tile([128, RUL], fp16, tag="fb", name="fb")
                        nc.scalar.copy(fb[:], ps[:])
                        fbs[i] = fb
                        if i in DVE_F:
                            ndve += 1
                            if ndve == 2:
                                nc.vector.tensor_mul(
                                    CA[:], fbs[DVE_F[0]][:], fbs[DVE_F[1]][:]
                                )
                            elif ndve > 2:
                                nc.vector.tensor_mul(CA[:], CA[:], fb[:])
                        else:
                            npool += 1
                            if npool == 2:
                                nc.gpsimd.tensor_mul(
                                    CP[:], fbs[POOL_F[0]][:], fbs[POOL_F[1]][:]
                                )
                            elif npool > 2:
                                nc.gpsimd.tensor_mul(CP[:], CP[:], fb[:])
                    if i == 4 and pending:
                        pending.pop(0)()

                def finalize(t=t, P=P, CA=CA, CP=CP):
                    nc.vector.tensor_mul(CA[:], CA[:], CP[:])
                    nc.vector.tensor_mul(P[:], P[:], CA[:])
                    osum = outpool.tile([128, ULOC], fp32, tag="osum", name="osum")
                    nc.vector.tensor_reduce(
                        osum[:],
                        P[:].rearrange("p (u r) -> p u r", r=R),
                        axis=mybir.AxisListType.X,
                        op=mybir.AluOpType.add,
                    )
                    nc.sync.dma_start(out_d[t * 128 : (t + 1) * 128, :], osum[:])

                pending.append(finalize)

            for fin in pending:
                fin()

    nc.compile()
    return nc


def _host_prep(X, K):
    """Repack inputs into per-core fp16 stationary/moving operands.

    Quad q covers features 4q..4q+3 as two K=128 PSUM-accumulated passes
    (row = ((d0*4+d1)*4+d2)*2 + l, l indexing half of the 4th feature's
    d range).  Triples cover features 48+3j..50+3j (row = d0*16+d1*4+d2),
    two per kt slot (rows 0:64 / 64:128) for row-tiled matmul pairs;
    feature 63 rides in the third pair's B half (rows 64:68).  Columns are
    u-major (col = u*32 + r).
    """
    f16 = np.float16
    FT = 4 * NQ                      # first triple feature
    kt_cores, xt_cores = [], []
    for bi in range(BSH):
        Xc = X[bi * BLOC : (bi + 1) * BLOC]                    # [512, 64, 4]
        for uj in range(USH):
            Ku = K[:, :, :, uj * ULOC : (uj + 1) * ULOC]       # [4,32,64,32]
            Kf = np.ascontiguousarray(
                Ku.transpose(2, 0, 3, 1).reshape(F, D, RUL)
            )                                                   # [f, d, col]
            kt = np.zeros((NSLOT, 128, RUL), dtype=f16)
            xt = np.zeros((NBT, 128, NSLOT * 128), dtype=f16)

            def put_x(slot, rows, arr):  # arr [BLOC, nrows]
                for t in range(NBT):
                    xt[t, rows, slot * 128 : (slot + 1) * 128] = arr[
                        t * 128 : (t + 1) * 128
                    ].T

            # triple pairs in slots 0..NPAIR-1
            for p in range(NPAIR):
                for s in range(2):
                    j = 2 * p + s
                    rows = slice(64 * s, 64 * s + 64)
                    f0 = FT + 3 * j
                    K3 = (
                        Kf[f0][:, None, None, :]
                        * Kf[f0 + 1][None, :, None, :]
                        * Kf[f0 + 2][None, None, :, :]
                    ).reshape(64, RUL)
                    X3 = (
                        Xc[:, f0, :, None, None]
                        * Xc[:, f0 + 1, None, :, None]
                        * Xc[:, f0 + 2, None, None, :]
                    ).reshape(BLOC, 64)
                    kt[p, rows] = K3
                    put_x(p, rows, X3)
            # quads in slots NPAIR + 2q + h
            for q in range(NQ):
                f0 = 4 * q
                K012 = (
                    Kf[f0][:, None, None, :]
                    * Kf[f0 + 1][None, :, None, :]
                    * Kf[f0 + 2][None, None, :, :]
                ).reshape(64, RUL)
                X012 = (
                    Xc[:, f0, :, None, None]
                    * Xc[:, f0 + 1, None, :, None]
                    * Xc[:, f0 + 2, None, None, :]
                ).reshape(BLOC, 64)
                for h in range(2):
                    slot = NPAIR + 2 * q + h
                    kt[slot] = (
                        K012[:, None, :] * Kf[f0 + 3][2 * h : 2 * h + 2][None, :, :]
                    ).reshape(128, RUL)
                    X4h = (
                        X012[:, :, None]
                        * Xc[:, f0 + 3, 2 * h : 2 * h + 2][:, None, :]
                    ).reshape(BLOC, 128)
                    put_x(slot, slice(0, 128), X4h)
            kt_cores.append(np.ascontiguousarray(kt))
            xt_cores.append(np.ascontiguousarray(xt))
    return [{"xt": xt_cores[c], "kt": kt_cores[c]} for c in range(NCORES)]


def kernel(**inputs):
    from concourse.bass_utils import run_bass_kernel_spmd

    X = np.asarray(inputs["X"], dtype=np.float32)
    K = np.asarray(inputs["kernel"], dtype=np.float32)
    assert X.shape == (B, F, D) and K.shape == (D, R, F, U)

    if "nc" not in _cached:
        _cached["nc"] = _build_nc()
    nc = _cached["nc"]

    in_maps = _host_prep(X, K)
    res = run_bass_kernel_spmd(nc, in_maps, core_ids=list(range(NCORES)))
    out = np.zeros((B, U), dtype=np.float32)
    for c in range(NCORES):
        bi, uj = divmod(c, USH)
        out[bi * BLOC : (bi + 1) * BLOC, uj * ULOC : (uj + 1) * ULOC] = res.results[
            c
        ]["out"]
    return out


# revision 11
# speedup vs baseline: 1.3381x; 1.1803x over previous
"""Trainium2 Bass kernel for nn_CP_Based (CP-decomposition interaction layer).

Math (full problem):
    t[b,f,r,u] = sum_d X[b,f,d] * K[d,r,f,u]      (B=1024, F=64, D=4, R=32, U=128)
    had[b,r,u] = prod_f t[b,f,r,u]
    out[b,u]   = sum_r had[b,r,u]

Strategy (v4):
  * Shard batch x units across 8 cores as (2 batch halves) x (4 unit
    quarters): per core B_loc=512 (4 partition tiles) and RU_loc = 32r x 32u
    = 1024 columns (u-major, r contiguous innermost for the final reduce).
  * Host-side feature grouping into multilinear factors: the PSUM-read
    bandwidth of the only two PSUM-capable engines (DVE @0.96 GHz, Act
    @1.2 GHz) is the hard roofline, so we minimize factor count.  12 QUADS
    of 4 features (K=4^4=256 as 2 PSUM-accumulated K=128 passes) + 5
    triples (K=64, row-tiled in pairs via tile_position) + feature 63
    alone = 18 factor tiles per batch tile instead of 64 features.
  * All matmul inputs fp16: the PE streams 512-row matmuls at ~214 ns when
    continuously busy (full p-state).  A 4-deep [128,1024] PSUM ring keeps
    it fed.
  * Drains: 5 tiles fold into an fp32 running product P on DVE (fused
    tensor_tensor with one PSUM operand - precision anchor); 13 tiles are
    drained to fp16 SBUF by Act.  DVE chains 8 of those (fp16 muls), Pool
    chains the other 5; final merges + the strided r-reduce on DVE.
"""

import numpy as np

B, F, D, R, U = 1024, 64, 4, 32, 128
NCORES = 8
BSH, USH = 2, 4                 # batch shards x unit shards
BLOC = B // BSH                 # 512 batch rows per core
NBT = BLOC // 128               # 4 batch tiles of 128
ULOC = U // USH                 # 32 units per core
RUL = R * ULOC                  # 1024 columns (u-major: col = u*32 + r)
NQ = 4                          # quads (features 0..15)
NT = 16                         # triples (features 16..63)
NTILE = NT + NQ                 # 20 factor tiles per batch tile
NPAIR = NT // 2                 # triple pairs
NSLOT = NPAIR + 2 * NQ          # kt slots: 8 triple-pairs + 2 per quad

# consumer assignment (tiles 0..5 = triple halves incl lone, 6..17 quads)
FUSED = (0, 3, 6, 9, 12, 15, 18)              # DVE fused fp32 chain (7)
ACTS = tuple(i for i in range(NTILE) if i not in FUSED)   # 13 Act drains
DVE_F = ACTS[0::2]                            # fp16 chain on DVE (7 tiles)
POOL_F = ACTS[1::2]                           # 6 tiles -> Pool chain (interleaved
                                              # so Pool works all btile long)

_cached = {}


def _build_nc():
    import concourse.bass as bass
    import concourse.mybir as mybir
    import concourse.tile as tile
    from concourse import bacc

    fp32 = mybir.dt.float32
    fp16 = mybir.dt.float16
    nc = bacc.Bacc("TRN2", target_bir_lowering=False, debug=False)

    xt_d = nc.dram_tensor("xt", [NBT, 128, NSLOT * 128], fp16, kind="ExternalInput").ap()
    kt_d = nc.dram_tensor("kt", [NSLOT, 128, RUL], fp16, kind="ExternalInput").ap()
    out_d = nc.dram_tensor("out", [BLOC, ULOC], fp32, kind="ExternalOutput").ap()

    with tile.TileContext(nc) as tc:
        with (
            tc.tile_pool(name="kt", bufs=1) as ktpool,
            tc.tile_pool(name="xt", bufs=1) as xtpool,
            tc.tile_pool(name="fb", bufs=10) as fbpool,
            tc.tile_pool(name="acc", bufs=2) as accpool,
            tc.tile_pool(name="out", bufs=2) as outpool,
            tc.tile_pool(name="ps", bufs=4, space="PSUM") as pspool,
        ):
            xts = []
            for t in range(NBT):
                xts.append(
                    xtpool.tile([128, NSLOT * 128], fp16, tag=f"xt{t}", name=f"xt{t}")
                )
            nc.sync.dma_start(xts[0][:], xt_d[0])
            kts = []
            for s in range(NSLOT):
                kts.append(ktpool.tile([128, RUL], fp16, tag=f"kt{s}", name=f"kt{s}"))
                nc.sync.dma_start(kts[s][:], kt_d[s])
            for t in range(1, NBT):
                nc.sync.dma_start(xts[t][:], xt_d[t])

            pending = []

            def xsl(s):
                return slice(s * 128, (s + 1) * 128)

            for t in range(NBT):
                xt = xts[t]
                P = accpool.tile([128, RUL], fp32, tag="P", name="P")
                CA = accpool.tile([128, RUL], fp16, tag="CA", name="CA")
                CP = accpool.tile([128, RUL], fp16, tag="CP", name="CP")
                fbs = {}
                nfused = 0
                ndve = 0
                npool = 0
                for i in range(NTILE):
                    ps = pspool.tile([128, RUL], fp32, tag="ps", name="ps")
                    if i < NT:  # triple half: pair p = i//2, sub s = i%2
                        p, s = divmod(i, 2)
                        rows = slice(64 * s, 64 * s + 64)
                        for h in range(2):
                            cs = slice(512 * h, 512 * h + 512)
                            nc.tensor.matmul(
                                ps[:, cs],
                                xt[rows, xsl(p)],
                                kts[p][rows, cs],
                                start=True,
                                stop=True,
                                tile_position=(64 * s, 0),
                            )
                    else:  # quad: 2 psum-accumulated K=128 passes
                        q = i - NT
                        for h in range(2):
                            slot = NPAIR + 2 * q + h
                            for c in range(2):
                                cs = slice(512 * c, 512 * c + 512)
                                nc.tensor.matmul(
                                    ps[:, cs],
                                    xt[:, xsl(slot)],
                                    kts[slot][:, cs],
                                    start=(h == 0),
                                    stop=(h == 1),
                                )
                    if i in FUSED:
                        nfused += 1
                        if nfused == 1:
                            nc.vector.tensor_copy(P[:], ps[:])
                        else:
                            nc.vector.tensor_mul(P[:], P[:], ps[:])
                    else:
                        fb = fbpool.tile([128, RUL], fp16, tag="fb", name="fb")
                        nc.scalar.copy(fb[:], ps[:])
                        fbs[i] = fb
                        if i in DVE_F:
                            ndve += 1
                            if ndve == 2:
                                nc.vector.tensor_mul(
                                    CA[:], fbs[DVE_F[0]][:], fbs[DVE_F[1]][:]
                                )
                            elif ndve > 2:
                                nc.vector.tensor_mul(CA[:], CA[:], fb[:])
                        else:
                            npool += 1
                            if npool == 2:
                                nc.gpsimd.tensor_mul(
                                    CP[:], fbs[POOL_F[0]][:], fbs[POOL_F[1]][:]
                                )
                            elif npool > 2:
                                nc.gpsimd.tensor_mul(CP[:], CP[:], fb[:])
                    if i == 6 and pending:
                        pending.pop(0)()

                def finalize(t=t, P=P, CA=CA, CP=CP):
                    nc.vector.tensor_mul(CA[:], CA[:], CP[:])
                    nc.vector.tensor_mul(P[:], P[:], CA[:])
                    osum = outpool.tile([128, ULOC], fp32, tag="osum", name="osum")
                    nc.vector.tensor_reduce(
                        osum[:],
                        P[:].rearrange("p (u r) -> p u r", r=R),
                        axis=mybir.AxisListType.X,
                        op=mybir.AluOpType.add,
                    )
                    nc.sync.dma_start(out_d[t * 128 : (t + 1) * 128, :], osum[:])

                pending.append(finalize)

            for fin in pending:
                fin()

    nc.compile()
    return nc


def _host_prep(X, K):
    """Repack inputs into per-core fp16 stationary/moving operands.

    Quad q covers features 4q..4q+3 as two K=128 PSUM-accumulated passes
    (row = ((d0*4+d1)*4+d2)*2 + l, l indexing half of the 4th feature's
    d range).  Triples cover features 48+3j..50+3j (row = d0*16+d1*4+d2),
    two per kt slot (rows 0:64 / 64:128) for row-tiled matmul pairs;
    feature 63 rides in the third pair's B half (rows 64:68).  Columns are
    u-major (col = u*32 + r).
    """
    f16 = np.float16
    FT = 4 * NQ                      # first triple feature
    kt_cores, xt_cores = [], []
    for bi in range(BSH):
        Xc = X[bi * BLOC : (bi + 1) * BLOC]                    # [512, 64, 4]
        for uj in range(USH):
            Ku = K[:, :, :, uj * ULOC : (uj + 1) * ULOC]       # [4,32,64,32]
            Kf = np.ascontiguousarray(
                Ku.transpose(2, 0, 3, 1).reshape(F, D, RUL)
            )                                                   # [f, d, col]
            kt = np.zeros((NSLOT, 128, RUL), dtype=f16)
            xt = np.zeros((NBT, 128, NSLOT * 128), dtype=f16)

            def put_x(slot, rows, arr):  # arr [BLOC, nrows]
                for t in range(NBT):
                    xt[t, rows, slot * 128 : (slot + 1) * 128] = arr[
                        t * 128 : (t + 1) * 128
                    ].T

            # triple pairs in slots 0..NPAIR-1
            for p in range(NPAIR):
                for s in range(2):
                    j = 2 * p + s
                    rows = slice(64 * s, 64 * s + 64)
                    f0 = FT + 3 * j
                    K3 = (
                        Kf[f0][:, None, None, :]
                        * Kf[f0 + 1][None, :, None, :]
                        * Kf[f0 + 2][None, None, :, :]
                    ).reshape(64, RUL)
                    X3 = (
                        Xc[:, f0, :, None, None]
                        * Xc[:, f0 + 1, None, :, None]
                        * Xc[:, f0 + 2, None, None, :]
                    ).reshape(BLOC, 64)
                    kt[p, rows] = K3
                    put_x(p, rows, X3)
            # quads in slots NPAIR + 2q + h
            for q in range(NQ):
                f0 = 4 * q
                K012 = (
                    Kf[f0][:, None, None, :]
                    * Kf[f0 + 1][None, :, None, :]
                    * Kf[f0 + 2][None, None, :, :]
                ).reshape(64, RUL)
                X012 = (
                    Xc[:, f0, :, None, None]
                    * Xc[:, f0 + 1, None, :, None]
                    * Xc[:, f0 + 2, None, None, :]
                ).reshape(BLOC, 64)
                for h in range(2):
                    slot = NPAIR + 2 * q + h
                    kt[slot] = (
                        K012[:, None, :] * Kf[f0 + 3][2 * h : 2 * h + 2][None, :, :]
                    ).reshape(128, RUL)
                    X4h = (
                        X012[:, :, None]
                        * Xc[:, f0 + 3, 2 * h : 2 * h + 2][:, None, :]
                    ).reshape(BLOC, 128)
                    put_x(slot, slice(0, 128), X4h)
            kt_cores.append(np.ascontiguousarray(kt))
            xt_cores.append(np.ascontiguousarray(xt))
    return [{"xt": xt_cores[c], "kt": kt_cores[c]} for c in range(NCORES)]


def kernel(**inputs):
    from concourse.bass_utils import run_bass_kernel_spmd

    X = np.asarray(inputs["X"], dtype=np.float32)
    K = np.asarray(inputs["kernel"], dtype=np.float32)
    assert X.shape == (B, F, D) and K.shape == (D, R, F, U)

    if "nc" not in _cached:
        _cached["nc"] = _build_nc()
    nc = _cached["nc"]

    in_maps = _host_prep(X, K)
    res = run_bass_kernel_spmd(nc, in_maps, core_ids=list(range(NCORES)))
    out = np.zeros((B, U), dtype=np.float32)
    for c in range(NCORES):
        bi, uj = divmod(c, USH)
        out[bi * BLOC : (bi + 1) * BLOC, uj * ULOC : (uj + 1) * ULOC] = res.results[
            c
        ]["out"]
    return out
